# revision 27
# baseline (speedup 1.0000x reference)
"""Trainium2 Bass kernel: Jaccard-similarity graph coarsening (pooling).

Pipeline (matches the jax reference bit-for-bit where it matters):
  1. Device (8 NeuronCores, SPMD): inter = A @ A.T, upper triangle only.
     A is 0/1 so fp8e4 inputs + fp32 PSUM accumulation give EXACT integer
     counts. Work split: 8 cores = 2 row-strip halves x 4 k-quarters.
     Each core reads only a 4 MiB k-slab of A (SBUF resident) and computes
     partial counts for its half of the 136 strip-pair blocks; the host
     sums the 4 k-partials (exact small integers in bf16) and mirrors.
     The two halves have different (but perfectly work-balanced: 32 N512 +
     4 N256 chunk-tasks each) static trapezoids, selected per-core with a
     tc.If on the partition id.
  2. Host: union/sim (fp32), sequential greedy union-find (inherently
     serial, done in numpy exactly like the reference), P construction and
     the tiny P.T@X / P.T@A@P projections replicated with default-platform
     jax.numpy so they match the reference's numerics.
"""

import numpy as np
import ml_dtypes

N = 4096
D = 256
NUM_SUPER_NODES = 6
THRESHOLD = 0.0
EPS = 1e-10
NCORES = 8
SW = 256                  # row-strip width (16 strips)
KQ = 4                    # k-quarters
KT_LOC = N // 128 // KQ   # 8 k-tiles per slab
KP_LOC = KT_LOC // 2      # 4 DoubleRow k-pairs per slab
FP8 = ml_dtypes.float8_e4m3

_CACHE = {}


def _sus(g):
    """Row strips of half g — balanced: both halves have 68 triu blocks
    and identical chunk-task width multisets (32 x 512 + 4 x 256)."""
    return [0, 1, 2, 3, 12, 13, 14, 15] if g == 0 else [4, 5, 6, 7, 8, 9, 10, 11]


def _half_tasks(g):
    """(su, rhs col, width) chunk-tasks covering cols >= su*SW. 36 per half."""
    tasks = []
    for su in _sus(g):
        c = su * SW
        if su % 2 == 1:
            tasks.append((su, c, 256))
            c += 256
        while c < N:
            tasks.append((su, c, 512))
            c += 512
    assert len(tasks) == 36
    # Cheap N256 tasks last: the kernel tail (last cast + out DMA before the
    # exit barrier) drains faster.
    return [t for t in tasks if t[2] == 512] + [t for t in tasks if t[2] == 256]


def _build_nc():
    """SPMD kernel with per-core-half trapezoids.

    Input (per core): a_slab [128, KT_LOC, N] fp8 — A rows of its k-quarter
    (4 MiB, SBUF resident). Output: o [36, SW, 512] bf16 partial counts.
    """
    import concourse.mybir as mybir
    from concourse import bacc, tile

    dt = mybir.dt
    nc = bacc.Bacc("TRN2", target_bir_lowering=False, debug=False)
    a_in = nc.dram_tensor("a_slab", [128, KT_LOC, N], dt.float8e4,
                          kind="ExternalInput")
    o_out = nc.dram_tensor("o", [36, SW, 512], dt.bfloat16, kind="ExternalOutput")

    with tile.TileContext(nc) as tc:
        pid = nc.partition_id()
        with (
            tc.tile_pool(name="abuf", bufs=1) as apool,
            tc.tile_pool(name="psum", bufs=8, space="PSUM") as pspool,
            tc.tile_pool(name="obuf", bufs=4) as opool,
        ):
            def body(g):
                # Input DMAs live inside the branch (a tc.If schedules as a
                # discrete unit, so outside DMAs would not overlap compute).
                chunks = []
                for ck in range(KP_LOC):
                    t = apool.tile([128, 2, N], dt.float8e4,
                                   tag=f"a{ck}", name=f"a{g}_{ck}")
                    eng = nc.sync if ck % 2 == 0 else nc.scalar
                    eng.dma_start(out=t[:], in_=a_in[:, 2 * ck : 2 * ck + 2, :])
                    chunks.append(t)

                for ti, (su, rcol, w) in enumerate(_half_tasks(g)):
                    lcol = su * SW
                    rot = ti % KP_LOC  # stagger starts across chunk arrivals
                    for mi in range(2):
                        ps = pspool.tile([128, w], dt.float32, tag="ps", name="ps")
                        for i in range(KP_LOC):
                            kk = (rot + i) % KP_LOC
                            nc.tensor.matmul(
                                ps[:],
                                lhsT=chunks[kk][:, 0:2, lcol + mi * 128 : lcol + mi * 128 + 128],
                                rhs=chunks[kk][:, 0:2, rcol : rcol + w],
                                start=(i == 0),
                                stop=(i == KP_LOC - 1),
                                perf_mode=mybir.MatmulPerfMode.DoubleRow,
                            )
                        ob = opool.tile([128, w], dt.bfloat16, tag="ob", name="ob")
                        nc.vector.tensor_copy(ob[:], ps[:])
                        eng = (nc.sync, nc.scalar, nc.gpsimd)[(2 * ti + mi) % 3]
                        eng.dma_start(
                            out=o_out[ti, mi * 128 : (mi + 1) * 128, 0:w], in_=ob[:]
                        )

            with tc.If(pid < 4) as cmp:
                body(0)
            with cmp.Else():
                body(1)

    nc.compile()
    if not nc.is_finalized():
        nc.finalize()
    return nc


def _get_nc():
    if "nc" not in _CACHE:
        _CACHE["nc"] = _build_nc()
    return _CACHE["nc"]


def _perm(a):
    """[rows, W] row-major -> [128, rows/128, W]: (p, k, j) = a[k*128+p, j]."""
    kt = a.shape[0] // 128
    w = a.shape[1]
    return np.ascontiguousarray(a.reshape(kt, 128, w).transpose(1, 0, 2))


def _device_inter(A8):
    """Run the SPMD Bass kernel on 8 cores; return full [N, N] fp32 inter."""
    from concourse.bass_utils import run_bass_kernel_spmd

    nc = _get_nc()
    kq_rows = N // KQ
    slabs = [
        _perm(np.ascontiguousarray(A8[h * kq_rows : (h + 1) * kq_rows, :]))
        for h in range(KQ)
    ]
    in_maps = [{"a_slab": slabs[c % KQ]} for c in range(NCORES)]
    res = run_bass_kernel_spmd(nc, in_maps, core_ids=list(range(NCORES)))
    _CACHE["last_results"] = res

    inter = np.empty((N, N), np.float32)
    for g in range(2):
        part = np.zeros((36, SW, 512), np.float32)
        for h in range(KQ):
            part += res.results[4 * g + h]["o"].astype(np.float32)
        for ti, (su, rcol, w) in enumerate(_half_tasks(g)):
            blk = part[ti, :, :w]
            inter[su * SW : (su + 1) * SW, rcol : rcol + w] = blk
            inter[rcol : rcol + w, su * SW : (su + 1) * SW] = blk.T
    return inter


def _greedy_labels(sim_np, n, num_super, threshold):
    """Verbatim reference implementation (sequential, data-dependent)."""
    iu, ju = np.triu_indices(n, k=1)
    sims = sim_np[iu, ju]
    keep = sims >= threshold
    iu, ju, sims = iu[keep], ju[keep], sims[keep]
    order = np.argsort(-sims, kind="stable")
    parent = np.arange(n)

    def find(i):
        while parent[i] != i:
            parent[i] = parent[parent[i]]
            i = parent[i]
        return i

    merged = 0
    for k in order:
        pi, pj = find(int(iu[k])), find(int(ju[k]))
        if pi != pj:
            parent[pj] = pi
            merged += 1
            if n - merged <= num_super:
                break
    mapping = {}
    labels = np.empty(n, dtype=np.int64)
    nl = 0
    for i in range(n):
        r = find(i)
        if r not in mapping:
            mapping[r] = nl
            nl += 1
        labels[i] = mapping[r]
    return labels, nl


def kernel(X, A):
    import jax
    import jax.numpy as jnp

    X_np = np.asarray(X, dtype=np.float32)
    A_np = np.asarray(A, dtype=np.float32)
    A8 = (A_np > 0).astype(FP8)  # 0/1 are exact in fp8e4

    inter = _device_inter(A8)

    # Downstream of the big matmul: replicate the reference ops on the
    # default jax platform so sim bits (and thus the greedy merge order)
    # match the reference run in the same environment.
    try:
        A_j = jnp.asarray(A_np)
        A_bin = (A_j > 0).astype(jnp.float32)
        deg = A_bin.sum(axis=1)
        inter_j = jnp.asarray(inter)
        union = deg[:, None] + deg[None, :] - inter_j
        sim = jnp.where(union > 0, inter_j / jnp.maximum(union, 1.0), 0.0)
        sim_np = np.asarray(sim)
    except Exception:
        deg_np = A_np.sum(axis=1, dtype=np.float32)
        union_np = (deg_np[:, None] + deg_np[None, :]) - inter
        sim_np = np.where(
            union_np > 0, inter / np.maximum(union_np, np.float32(1.0)), np.float32(0.0)
        ).astype(np.float32)

    labels, m = _greedy_labels(sim_np, N, NUM_SUPER_NODES, THRESHOLD)

    try:
        P_prime = jax.nn.one_hot(jnp.asarray(labels), m, dtype=jnp.float32)
        sizes = P_prime.sum(axis=0)
        P = P_prime / jnp.sqrt(sizes + EPS)[None, :]
        X_coarse = P.T @ jnp.asarray(X_np)
        A_coarse = P.T @ jnp.asarray(A_np) @ P
        return (np.asarray(X_coarse), np.asarray(A_coarse), np.asarray(P))
    except Exception:
        P_prime = np.zeros((N, m), np.float32)
        P_prime[np.arange(N), labels] = 1.0
        sizes = P_prime.sum(axis=0)
        P = P_prime / np.sqrt(sizes + np.float32(EPS))[None, :]
        X_coarse = P.T @ X_np
        A_coarse = P.T @ A_np @ P
        return (X_coarse, A_coarse, P)


# revision 29
# speedup vs baseline: 1.0440x; 1.0440x over previous
"""Trainium2 Bass kernel: Jaccard-similarity graph coarsening (pooling).

Pipeline (matches the jax reference bit-for-bit where it matters):
  1. Device (8 NeuronCores, SPMD): inter = A @ A.T, upper triangle only.
     A is 0/1 so fp8e4 inputs + fp32 PSUM accumulation give EXACT integer
     counts. Work split: 8 cores = 2 row-strip halves x 4 k-quarters.
     Each core reads only a 4 MiB k-slab of A (SBUF resident) and computes
     partial counts for its half of the 136 strip-pair blocks; the host
     sums the 4 k-partials (exact small integers in bf16) and mirrors.
     The two halves have different (but perfectly work-balanced: 32 N512 +
     4 N256 chunk-tasks each) static trapezoids, selected per-core with a
     tc.If on the partition id.
  2. Host: union/sim (fp32), sequential greedy union-find (inherently
     serial, done in numpy exactly like the reference), P construction and
     the tiny P.T@X / P.T@A@P projections replicated with default-platform
     jax.numpy so they match the reference's numerics.
"""

import numpy as np
import ml_dtypes

N = 4096
D = 256
NUM_SUPER_NODES = 6
THRESHOLD = 0.0
EPS = 1e-10
NCORES = 8
SW = 256                  # row-strip width (16 strips)
KQ = 4                    # k-quarters
KT_LOC = N // 128 // KQ   # 8 k-tiles per slab
KP_LOC = KT_LOC // 2      # 4 DoubleRow k-pairs per slab
FP8 = ml_dtypes.float8_e4m3

_CACHE = {}


def _sus(g):
    """Row strips of half g — balanced: both halves have 68 triu blocks
    and identical chunk-task width multisets (32 x 512 + 4 x 256)."""
    return [0, 1, 2, 3, 12, 13, 14, 15] if g == 0 else [4, 5, 6, 7, 8, 9, 10, 11]


def _half_tasks(g):
    """(su, rhs col, width) chunk-tasks covering cols >= su*SW. 36 per half."""
    tasks = []
    for su in _sus(g):
        c = su * SW
        if su % 2 == 1:
            tasks.append((su, c, 256))
            c += 256
        while c < N:
            tasks.append((su, c, 512))
            c += 512
    assert len(tasks) == 36
    return tasks


def _build_nc():
    """SPMD kernel with per-core-half trapezoids.

    Input (per core): a_slab [128, KT_LOC, N] fp8 — A rows of its k-quarter
    (4 MiB, SBUF resident). Output: o [36, SW, 512] bf16 partial counts.
    """
    import concourse.mybir as mybir
    from concourse import bacc, tile

    dt = mybir.dt
    nc = bacc.Bacc("TRN2", target_bir_lowering=False, debug=False)
    a_in = nc.dram_tensor("a_slab", [128, KT_LOC, N], dt.float8e4,
                          kind="ExternalInput")
    o_out = nc.dram_tensor("o", [36, SW, 512], dt.bfloat16, kind="ExternalOutput")

    with tile.TileContext(nc) as tc:
        pid = nc.partition_id()
        with (
            tc.tile_pool(name="abuf", bufs=1) as apool,
            tc.tile_pool(name="psum", bufs=8, space="PSUM") as pspool,
            tc.tile_pool(name="obuf", bufs=4) as opool,
        ):
            def body(g):
                # Input DMAs live inside the branch (a tc.If schedules as a
                # discrete unit, so outside DMAs would not overlap compute).
                chunks = []
                for ck in range(KP_LOC):
                    t = apool.tile([128, 2, N], dt.float8e4,
                                   tag=f"a{ck}", name=f"a{g}_{ck}")
                    eng = nc.sync if ck % 2 == 0 else nc.scalar
                    eng.dma_start(out=t[:], in_=a_in[:, 2 * ck : 2 * ck + 2, :])
                    chunks.append(t)

                for ti, (su, rcol, w) in enumerate(_half_tasks(g)):
                    lcol = su * SW
                    rot = ti % KP_LOC  # stagger starts across chunk arrivals
                    for mi in range(2):
                        ps = pspool.tile([128, w], dt.float32, tag="ps", name="ps")
                        for i in range(KP_LOC):
                            kk = (rot + i) % KP_LOC
                            nc.tensor.matmul(
                                ps[:],
                                lhsT=chunks[kk][:, 0:2, lcol + mi * 128 : lcol + mi * 128 + 128],
                                rhs=chunks[kk][:, 0:2, rcol : rcol + w],
                                start=(i == 0),
                                stop=(i == KP_LOC - 1),
                                perf_mode=mybir.MatmulPerfMode.DoubleRow,
                            )
                        ob = opool.tile([128, w], dt.bfloat16, tag="ob", name="ob")
                        nc.vector.tensor_copy(ob[:], ps[:])
                        eng = nc.sync if ti % 2 == 0 else nc.scalar
                        eng.dma_start(
                            out=o_out[ti, mi * 128 : (mi + 1) * 128, 0:w], in_=ob[:]
                        )

            with tc.If(pid < 4) as cmp:
                body(0)
            with cmp.Else():
                body(1)

    nc.compile()
    if not nc.is_finalized():
        nc.finalize()
    return nc


def _get_nc():
    if "nc" not in _CACHE:
        _CACHE["nc"] = _build_nc()
    return _CACHE["nc"]


def _perm(a):
    """[rows, W] row-major -> [128, rows/128, W]: (p, k, j) = a[k*128+p, j]."""
    kt = a.shape[0] // 128
    w = a.shape[1]
    return np.ascontiguousarray(a.reshape(kt, 128, w).transpose(1, 0, 2))


def _device_inter(A8):
    """Run the SPMD Bass kernel on 8 cores; return full [N, N] fp32 inter."""
    from concourse.bass_utils import run_bass_kernel_spmd

    nc = _get_nc()
    kq_rows = N // KQ
    slabs = [
        _perm(np.ascontiguousarray(A8[h * kq_rows : (h + 1) * kq_rows, :]))
        for h in range(KQ)
    ]
    in_maps = [{"a_slab": slabs[c % KQ]} for c in range(NCORES)]
    res = run_bass_kernel_spmd(nc, in_maps, core_ids=list(range(NCORES)))
    _CACHE["last_results"] = res

    inter = np.empty((N, N), np.float32)
    for g in range(2):
        part = np.zeros((36, SW, 512), np.float32)
        for h in range(KQ):
            part += res.results[4 * g + h]["o"].astype(np.float32)
        for ti, (su, rcol, w) in enumerate(_half_tasks(g)):
            blk = part[ti, :, :w]
            inter[su * SW : (su + 1) * SW, rcol : rcol + w] = blk
            inter[rcol : rcol + w, su * SW : (su + 1) * SW] = blk.T
    return inter


def _greedy_labels(sim_np, n, num_super, threshold):
    """Verbatim reference implementation (sequential, data-dependent)."""
    iu, ju = np.triu_indices(n, k=1)
    sims = sim_np[iu, ju]
    keep = sims >= threshold
    iu, ju, sims = iu[keep], ju[keep], sims[keep]
    order = np.argsort(-sims, kind="stable")
    parent = np.arange(n)

    def find(i):
        while parent[i] != i:
            parent[i] = parent[parent[i]]
            i = parent[i]
        return i

    merged = 0
    for k in order:
        pi, pj = find(int(iu[k])), find(int(ju[k]))
        if pi != pj:
            parent[pj] = pi
            merged += 1
            if n - merged <= num_super:
                break
    mapping = {}
    labels = np.empty(n, dtype=np.int64)
    nl = 0
    for i in range(n):
        r = find(i)
        if r not in mapping:
            mapping[r] = nl
            nl += 1
        labels[i] = mapping[r]
    return labels, nl


def kernel(X, A):
    import jax
    import jax.numpy as jnp

    X_np = np.asarray(X, dtype=np.float32)
    A_np = np.asarray(A, dtype=np.float32)
    A8 = (A_np > 0).astype(FP8)  # 0/1 are exact in fp8e4

    inter = _device_inter(A8)

    # Downstream of the big matmul: replicate the reference ops on the
    # default jax platform so sim bits (and thus the greedy merge order)
    # match the reference run in the same environment.
    try:
        A_j = jnp.asarray(A_np)
        A_bin = (A_j > 0).astype(jnp.float32)
        deg = A_bin.sum(axis=1)
        inter_j = jnp.asarray(inter)
        union = deg[:, None] + deg[None, :] - inter_j
        sim = jnp.where(union > 0, inter_j / jnp.maximum(union, 1.0), 0.0)
        sim_np = np.asarray(sim)
    except Exception:
        deg_np = A_np.sum(axis=1, dtype=np.float32)
        union_np = (deg_np[:, None] + deg_np[None, :]) - inter
        sim_np = np.where(
            union_np > 0, inter / np.maximum(union_np, np.float32(1.0)), np.float32(0.0)
        ).astype(np.float32)

    labels, m = _greedy_labels(sim_np, N, NUM_SUPER_NODES, THRESHOLD)

    try:
        P_prime = jax.nn.one_hot(jnp.asarray(labels), m, dtype=jnp.float32)
        sizes = P_prime.sum(axis=0)
        P = P_prime / jnp.sqrt(sizes + EPS)[None, :]
        X_coarse = P.T @ jnp.asarray(X_np)
        A_coarse = P.T @ jnp.asarray(A_np) @ P
        return (np.asarray(X_coarse), np.asarray(A_coarse), np.asarray(P))
    except Exception:
        P_prime = np.zeros((N, m), np.float32)
        P_prime[np.arange(N), labels] = 1.0
        sizes = P_prime.sum(axis=0)
        P = P_prime / np.sqrt(sizes + np.float32(EPS))[None, :]
        X_coarse = P.T @ X_np
        A_coarse = P.T @ A_np @ P
        return (X_coarse, A_coarse, P)
